# revision 25
# baseline (speedup 1.0000x reference)
"""Trainium2 Bass kernel for nn_BinaryMemory (retrieval_knn).

reference:
    gated = sigmoid(query @ W.T + b)                      # [1, D], D=4096
    sims  = 1 - mean(|memory - gated|, axis=-1)           # [N],   N=16384
    mask  = sims >= 0.8

Sharding (8 cores, no collectives): shard the D axis; core c owns
d-chunk [c*512, (c+1)*512). All bulk tensors stream as fp8_e3m4 (1 byte,
4 mantissa bits; operands live in (0,1) or N(0,1), quantization puts
~1e-2 relative on sims vs the 2e-2 budget). Per-core HBM ~10.1 MB; one
HWDGE ring sustains ~320 GB/s (a second ring adds nothing - per-core
HBM limit), so DMA floors the kernel at ~33 us + init.

Layout is d-on-partitions (memory shard transposed host-side to
[512 d, 16384 n]) so the gate value g[d] is a per-partition scalar.
The elementwise work splits |m-g| = (m-g) - 2*min(m-g, 0):
  - DVE tiles: ONE stock pass, tensor_scalar(op0=subtract scalar1=g,
    op1=min scalar2=0) -> min-term (fp8 in/out hits the 2x port mode,
    ~2.26 us per [128,4096]; a fused sub+abs is not expressible: the
    ISA rejects abs_max/bitwise op1 pairings and accum_out forces 1x).
  - ScalarE tiles: one Abs activation with per-partition bias -g
    (|m-g| directly, ~3.6 us/tile, dtype-agnostic).
The n-reduction runs on the otherwise-idle PE: per 512-column group a
psum row accumulates ones^T @ m (issued as soon as the tile lands - no
gate dependency) plus (-2*ones)^T @ minterm for DVE tiles, or
ones^T @ |m-g| for ScalarE tiles. The spurious +sum(g) from the m-term
is cancelled on the host via per-d-chunk gate sums (psum row 32 of the
output). Each psum bank holds 4 group-rows at quadrant offsets
{0,32,64,96} (tile_position); one [128,512] psum->SBUF copy drains 4
groups and a partition-strided DMA writes the rows to DRAM on the
otherwise-idle SWDGE (gpsimd) ring.

Gate: host pre-packs W-shard^T as [128, 32*512] fp8, DMA'd first on the
sync ring in 4 chunk-tiles so matmuls start as chunks land. q rides as
32 stationary columns; matmul j targets quadrant strip 32*(j%4)
(tile_position) so the PE's reorder window pulls each LDWEIGHTS ahead
of the in-flight MATMUL on the neighbouring strip - the serial LDW+MM
turnaround (~300 ns) drops to pipelined rate. The 4 partial z-rows are
copied out of psum, re-scattered to per-partition columns by a
transposed-AP SBUF->SBUF DMA, summed on DVE ([128,4,4] X-reduce),
biased with b, sigmoided on [128,4], and negated - no PE transpose, no
[1,512] row ops. Activation tables preload via dummy ops at t0, and
the small constants ride in two packed dram tensors (2 descriptors so
the scalar ring frees up early).
"""
import sys

sys.path.insert(0, "/opt/trn_rl_repo")

import numpy as np
import ml_dtypes

import concourse.bacc as bacc
import concourse.mybir as mybir
import concourse.tile as tile
from concourse.bass_utils import run_bass_kernel_spmd

N_CORES = 8
D = 4096
N = 16384
D_SH = D // N_CORES            # 512 dims per core
DC = D_SH // 128               # 4 d-chunks (partition blocks)
NT = 4096                      # n per tile
NK = N // NT                   # 4 n-chunks
NG = NT // 512                 # 8 psum groups per tile
THRESHOLD = 0.8
# tiles (k, c) computed on ScalarE via Abs; the rest on DVE via sub+min
ACT_TILES = {(0, 2), (1, 1), (1, 3), (2, 0), (3, 1), (3, 2)}
F16_TILES = [(0, 1), (0, 3)]   # DVE tiles streamed as fp16
DVE_SETS = [
    [c for c in range(DC) if (k, c) not in ACT_TILES] for k in range(NK)
]

_CACHE = {}


def _build():
    f32 = mybir.dt.float32
    f8 = mybir.dt.float8e3
    A = mybir.AluOpType
    AF = mybir.ActivationFunctionType
    nc = bacc.Bacc(
        "TRN2", target_bir_lowering=False, debug=False, num_devices=N_CORES
    )

    memT = nc.dram_tensor("memT", [D_SH, N], f8, kind="ExternalInput")
    memT16 = (
        nc.dram_tensor(
            "memT16", [len(F16_TILES) * 128, NT], mybir.dt.float16,
            kind="ExternalInput",
        )
        if F16_TILES
        else None
    )
    # W shard, host-packed: partition p, chunk j holds W.T[j*128 + p, :]
    wtp = nc.dram_tensor("wtp", [128, 32 * D_SH], f8, kind="ExternalInput")
    # packed constants: cols 0:32 qcol, 32 ones, 33 neg2, 34 selsum
    c8 = nc.dram_tensor("c8", [128, 35], f8, kind="ExternalInput")
    # packed f32 constants: cols 0:4 b columns, 4 ones, 5:9 eye(4),
    # 9 selsum (1.0 at partitions {0,32,64,96})
    c32 = nc.dram_tensor("c32", [128, 10], f32, kind="ExternalInput")
    outp = nc.dram_tensor("outp", [33, 512], f32, kind="ExternalOutput")

    with tile.TileContext(nc) as tc:
        with (
            tc.tile_pool(name="wts", bufs=1) as wpool,
            tc.tile_pool(name="mem", bufs=10) as mpool,
            tc.tile_pool(name="dts", bufs=3) as dpool,
            tc.tile_pool(name="acts", bufs=2) as apool,
            tc.tile_pool(name="cp", bufs=8) as cppool,
            tc.tile_pool(name="small", bufs=1) as spool,
            tc.tile_pool(name="psg", bufs=1, space="PSUM") as ppg,
            tc.tile_pool(name="psm", bufs=7, space="PSUM") as ppm,
        ):
            # gate weights first on the sync ring (mem stream queues
            # behind); 2 packed constant tensors on the scalar ring.
            wts = []
            for h in range(8):
                wt_sb = wpool.tile([128, 4 * D_SH], f8, tag=f"wt{h}")
                nc.sync.dma_start(
                    out=wt_sb[:],
                    in_=wtp[:, h * 4 * D_SH : (h + 1) * 4 * D_SH],
                )
                wts.append(wt_sb)
            c8_sb = spool.tile([128, 35], f8, tag="c8")
            nc.scalar.dma_start(out=c8_sb[:], in_=c8[:])
            c32_sb = spool.tile([128, 10], f32, tag="c32")
            nc.scalar.dma_start(out=c32_sb[:], in_=c32[:])
            qc_sb = c8_sb[:, 0:32]
            ones_sb = c8_sb[:, 32:33]
            neg2_sb = c8_sb[:, 33:34]
            selsum8 = c8_sb[:, 34:35]
            b4 = c32_sb[:, 0:4]
            ones32_sb = c32_sb[:, 4:5]
            id4 = c32_sb[0:4, 5:9]
            selsum = c32_sb[:, 9:10]
            # preload Sigmoid+Abs activation tables off the critical path
            dum = spool.tile([1, 4], f32, tag="dum")
            nc.scalar.activation(dum[:], c32_sb[0:1, 0:4], AF.Sigmoid)
            nc.scalar.activation(dum[:], c32_sb[0:1, 0:4], AF.Abs)

            # ---- gate: 4 quadrant strips accumulate partial z rows ----
            zps = ppg.tile([128, D_SH], f32, tag="z")
            # zero the bank: zcp/selsum read all 128 partitions and stale
            # psum bits can be NaN (NaN*0 = NaN would poison the gate)
            nc.vector.memset(zps[:], 0.0)
            for j in range(32):
                r = j % 4
                nc.tensor.matmul(
                    zps[32 * r : 32 * r + 1, :],
                    qc_sb[:, j : j + 1],
                    wts[j // 4][:, (j % 4) * D_SH : (j % 4 + 1) * D_SH],
                    start=(j < 4),
                    stop=(j >= 28),
                    tile_position=(0, 32 * r),
                    skip_group_check=True,
                )
            hp = tc.high_priority()
            hp.__enter__()
            zcp = spool.tile([128, D_SH], mybir.dt.float16, tag="zcp")
            nc.scalar.activation(zcp[:], zps[:], AF.Copy)
            # strip-sum on the PE (selsum picks rows {0,32,64,96}), then
            # 4 row->column transposes; no SBUF<->SBUF DMA hop
            nc.tensor.matmul(
                zps[0:1, :], selsum8, zcp[:], start=True, stop=True,
                skip_group_check=True,
            )
            zrow = spool.tile([1, D_SH], f32, tag="zrow")
            nc.vector.tensor_copy(zrow[:], zps[0:1, :])
            tps = zps[:, 504:508]
            for c in range(DC):
                nc.tensor.transpose(
                    tps[:, c : c + 1],
                    zrow[0:1, c * 128 : (c + 1) * 128],
                    ones32_sb[0:1, 0:1],
                )
            zb = spool.tile([128, DC], f32, tag="zb")
            nc.vector.tensor_tensor(zb[:], tps, b4, A.add)
            gpos = spool.tile([128, DC], f32, tag="gpos")
            nc.scalar.activation(gpos[:], zb[:], AF.Sigmoid)
            negg = spool.tile([128, DC], f32, tag="negg")
            nc.vector.tensor_scalar(negg[:], gpos[:], -1.0, None, A.mult)
            hp.__exit__(None, None, None)

            # ---- main loop ----
            # phase 1 issues the gate-independent m-term matmuls right at
            # each tile's DMA: the PE consumes them in the idle gate
            # window, never overlapping the DVE/ScalarE pass on the same
            # tile (same-tile SBUF contention costs ~20%).
            for k in range(NK):
                bank0 = ppm.tile([128, 512], f32, tag="bank")
                bank1 = ppm.tile([128, 512], f32, tag="bank")
                banks = [bank0, bank1]
                total_passes = sum(
                    1 if (k, c) in ACT_TILES else 2 for c in range(DC)
                )
                seen = [0] * NG
                mts = []
                for c in range(DC):
                    if (k, c) in F16_TILES:
                        i16 = F16_TILES.index((k, c))
                        mt = mpool.tile(
                            [128, NT], mybir.dt.float16, tag="m16"
                        )
                        nc.sync.dma_start(
                            out=mt[:],
                            in_=memT16[i16 * 128 : (i16 + 1) * 128, :],
                        )
                    else:
                        mt = mpool.tile([128, NT], f8, tag="m")
                        nc.sync.dma_start(
                            out=mt[:],
                            in_=memT[
                                c * 128 : (c + 1) * 128,
                                k * NT : (k + 1) * NT,
                            ],
                        )
                    mts.append(mt)
                    if (k, c) not in ACT_TILES:
                        for j in range(NG):
                            nc.tensor.matmul(
                                banks[j // 4][32 * (j % 4) : 32 * (j % 4) + 1, :],
                                ones_sb,
                                mt[:, j * 512 : (j + 1) * 512],
                                start=(seen[j] == 0),
                                stop=(seen[j] == total_passes - 1),
                                tile_position=(0, 32 * (j % 4)),
                                skip_group_check=True,
                            )
                            seen[j] += 1
                for c in range(DC):
                    mt = mts[c]
                    if (k, c) in ACT_TILES:
                        at = apool.tile([128, NT], f8, tag="a")
                        nc.scalar.activation(
                            at[:], mt[:], AF.Abs, bias=negg[:, c : c + 1]
                        )
                        src_, stat = at, ones_sb
                    else:
                        dt = dpool.tile(
                            [128, NT],
                            mybir.dt.float16 if (k, c) in F16_TILES else f8,
                            tag="d16" if (k, c) in F16_TILES else "d",
                        )
                        hv = 2 if (k, c) == (NK - 1, DC - 1) else 1
                        for v in range(hv):
                            sl_v = slice(v * NT // hv, (v + 1) * NT // hv)
                            nc.vector.tensor_scalar(
                                dt[:, sl_v], mt[:, sl_v],
                                gpos[:, c : c + 1], 0.0,
                                A.subtract, A.min,
                            )
                        src_, stat = dt, neg2_sb
                    for j in range(NG):
                        nc.tensor.matmul(
                            banks[j // 4][32 * (j % 4) : 32 * (j % 4) + 1, :],
                            stat,
                            src_[:, j * 512 : (j + 1) * 512],
                            start=(seen[j] == 0),
                            stop=(seen[j] == total_passes - 1),
                            tile_position=(0, 32 * (j % 4)),
                            skip_group_check=True,
                        )
                        seen[j] += 1
                for h in range(2):
                    cp = cppool.tile([128, 512], f32, tag="cp")
                    if h == 0:
                        nc.vector.tensor_copy(cp[:], banks[h][:])
                    else:
                        nc.scalar.activation(cp[:], banks[h][:], AF.Copy)
                    nc.gpsimd.dma_start(
                        out=outp[8 * k + 4 * h : 8 * k + 4 * h + 4, :],
                        in_=cp[0:128:32, :],
                    )

            # per-d-chunk gate sums for the host-side m-term correction;
            # issued last so the PE queue never stalls on gpos mid-loop
            gs = zps[0:1, 500:504]
            nc.tensor.matmul(
                gs, ones32_sb, gpos[:], start=True, stop=True,
                skip_group_check=True,
            )
            gs_sb = spool.tile([1, DC], f32, tag="gs")
            nc.vector.tensor_copy(gs_sb[:], gs)
            nc.gpsimd.dma_start(out=outp[32:33, 0:DC], in_=gs_sb[:])

    nc.compile()
    return nc


def _get_nc():
    if "nc" not in _CACHE:
        _CACHE["nc"] = _build()
    return _CACHE["nc"]


def kernel(query, W, b, memory, _trace=False, _return_raw=False):
    f8 = ml_dtypes.float8_e3m4
    query = np.asarray(query, dtype=np.float32)
    W = np.asarray(W, dtype=np.float32)
    b = np.asarray(b, dtype=np.float32)
    memory = np.asarray(memory, dtype=np.float32)

    mem8T = np.ascontiguousarray(memory.astype(f8).T)       # [D, N] fp8
    W8 = W.astype(f8)
    q8 = query.reshape(32, 128).astype(f8).T                # [128, 32]
    c8 = np.zeros((128, 35), dtype=f8)
    c8[:, 0:32] = q8
    c8[:, 32] = f8(1.0)
    c8[:, 33] = f8(-2.0)
    c8[0:128:32, 34] = f8(1.0)

    in_maps = []
    for c in range(N_CORES):
        sl = slice(c * D_SH, (c + 1) * D_SH)
        # wtp[p, j*512 + n] = W.T[j*128 + p, n] = W8[sl][n, j*128+p]
        wsh = W8[sl, :]                       # [512, 4096]
        wtp = np.ascontiguousarray(
            wsh.T.reshape(32, 128, D_SH).transpose(1, 0, 2).reshape(128, -1)
        )
        c32 = np.zeros((128, 10), dtype=np.float32)
        c32[:, 0:4] = b[sl].reshape(4, 128).T
        c32[:, 4] = 1.0
        c32[0:4, 5:9] = np.eye(4, dtype=np.float32)
        c32[0:128:32, 9] = 1.0
        m16 = np.empty((len(F16_TILES) * 128, NT), dtype=np.float16)
        for i16, (tk, tc) in enumerate(F16_TILES):
            m16[i16 * 128 : (i16 + 1) * 128, :] = memory[
                tk * NT : (tk + 1) * NT,
                c * D_SH + tc * 128 : c * D_SH + (tc + 1) * 128,
            ].T.astype(np.float16)
        in_maps.append(
            {
                "memT": np.ascontiguousarray(mem8T[sl, :]),
                **({"memT16": m16} if F16_TILES else {}),
                "wtp": wtp,
                "c8": c8,
                "c32": c32,
            }
        )

    nc = _get_nc()
    res = run_bass_kernel_spmd(
        nc, in_maps, list(range(N_CORES)), trace=_trace
    )

    total = np.zeros(N, dtype=np.float64)
    for c in range(N_CORES):
        out = res.results[c]["outp"]
        gsum = out[32, 0:DC].astype(np.float64)   # sum of g per d-chunk
        rows = out[0:32].reshape(NK, NG, 512)
        corr = np.array(
            [sum(gsum[ci] for ci in DVE_SETS[k]) for k in range(NK)]
        )
        total += (rows - corr[:, None, None]).reshape(N)
    sims = (1.0 - total / D).astype(np.float32)
    mask = sims >= THRESHOLD
    if _return_raw:
        return (sims, mask), res
    return sims, mask


# revision 26
# speedup vs baseline: 1.0536x; 1.0536x over previous
"""Trainium2 Bass kernel for nn_BinaryMemory (retrieval_knn).

reference:
    gated = sigmoid(query @ W.T + b)                      # [1, D], D=4096
    sims  = 1 - mean(|memory - gated|, axis=-1)           # [N],   N=16384
    mask  = sims >= 0.8

Sharding (8 cores, no collectives): shard the D axis; core c owns
d-chunk [c*512, (c+1)*512). All bulk tensors stream as fp8_e3m4 (1 byte,
4 mantissa bits; operands live in (0,1) or N(0,1), quantization puts
~1e-2 relative on sims vs the 2e-2 budget). Per-core HBM ~10.1 MB; one
HWDGE ring sustains ~320 GB/s (a second ring adds nothing - per-core
HBM limit), so DMA floors the kernel at ~33 us + init.

Layout is d-on-partitions (memory shard transposed host-side to
[512 d, 16384 n]) so the gate value g[d] is a per-partition scalar.
The elementwise work splits |m-g| = (m-g) - 2*min(m-g, 0):
  - DVE tiles: ONE stock pass, tensor_scalar(op0=subtract scalar1=g,
    op1=min scalar2=0) -> min-term (fp8 in/out hits the 2x port mode,
    ~2.26 us per [128,4096]; a fused sub+abs is not expressible: the
    ISA rejects abs_max/bitwise op1 pairings and accum_out forces 1x).
  - ScalarE tiles: one Abs activation with per-partition bias -g
    (|m-g| directly, ~3.6 us/tile, dtype-agnostic).
The n-reduction runs on the otherwise-idle PE: per 512-column group a
psum row accumulates ones^T @ m (issued as soon as the tile lands - no
gate dependency) plus (-2*ones)^T @ minterm for DVE tiles, or
ones^T @ |m-g| for ScalarE tiles. The spurious +sum(g) from the m-term
is cancelled on the host via per-d-chunk gate sums (psum row 32 of the
output). Each psum bank holds 4 group-rows at quadrant offsets
{0,32,64,96} (tile_position); one [128,512] psum->SBUF copy drains 4
groups and a partition-strided DMA writes the rows to DRAM on the
otherwise-idle SWDGE (gpsimd) ring.

Gate: host pre-packs W-shard^T as [128, 32*512] fp8, DMA'd first on the
sync ring in 4 chunk-tiles so matmuls start as chunks land. q rides as
32 stationary columns; matmul j targets quadrant strip 32*(j%4)
(tile_position) so the PE's reorder window pulls each LDWEIGHTS ahead
of the in-flight MATMUL on the neighbouring strip - the serial LDW+MM
turnaround (~300 ns) drops to pipelined rate. The 4 partial z-rows are
copied out of psum, re-scattered to per-partition columns by a
transposed-AP SBUF->SBUF DMA, summed on DVE ([128,4,4] X-reduce),
biased with b, sigmoided on [128,4], and negated - no PE transpose, no
[1,512] row ops. Activation tables preload via dummy ops at t0, and
the small constants ride in two packed dram tensors (2 descriptors so
the scalar ring frees up early).
"""
import sys

sys.path.insert(0, "/opt/trn_rl_repo")

import numpy as np
import ml_dtypes

import concourse.bacc as bacc
import concourse.mybir as mybir
import concourse.tile as tile
from concourse.bass_utils import run_bass_kernel_spmd

N_CORES = 8
D = 4096
N = 16384
D_SH = D // N_CORES            # 512 dims per core
DC = D_SH // 128               # 4 d-chunks (partition blocks)
NT = 4096                      # n per tile
NK = N // NT                   # 4 n-chunks
NG = NT // 512                 # 8 psum groups per tile
THRESHOLD = 0.8
# tiles (k, c) computed on ScalarE via Abs; the rest on DVE via sub+min
ACT_TILES = {(0, 2), (1, 1), (1, 3), (2, 0), (3, 1), (3, 2)}
F16_TILES = [(3, 0), (3, 3)]   # DVE tiles streamed as fp16
DVE_SETS = [
    [c for c in range(DC) if (k, c) not in ACT_TILES] for k in range(NK)
]

_CACHE = {}


def _build():
    f32 = mybir.dt.float32
    f8 = mybir.dt.float8e3
    A = mybir.AluOpType
    AF = mybir.ActivationFunctionType
    nc = bacc.Bacc(
        "TRN2", target_bir_lowering=False, debug=False, num_devices=N_CORES
    )

    memT = nc.dram_tensor("memT", [D_SH, N], f8, kind="ExternalInput")
    memT16 = (
        nc.dram_tensor(
            "memT16", [len(F16_TILES) * 128, NT], mybir.dt.float16,
            kind="ExternalInput",
        )
        if F16_TILES
        else None
    )
    # W shard, host-packed: partition p, chunk j holds W.T[j*128 + p, :]
    wtp = nc.dram_tensor("wtp", [128, 32 * D_SH], f8, kind="ExternalInput")
    # packed constants: cols 0:32 qcol, 32 ones, 33 neg2, 34 selsum
    c8 = nc.dram_tensor("c8", [128, 35], f8, kind="ExternalInput")
    # packed f32 constants: cols 0:4 b columns, 4 ones, 5:9 eye(4),
    # 9 selsum (1.0 at partitions {0,32,64,96})
    c32 = nc.dram_tensor("c32", [128, 10], f32, kind="ExternalInput")
    outp = nc.dram_tensor("outp", [33, 512], f32, kind="ExternalOutput")

    with tile.TileContext(nc) as tc:
        with (
            tc.tile_pool(name="wts", bufs=1) as wpool,
            tc.tile_pool(name="mem", bufs=10) as mpool,
            tc.tile_pool(name="dts", bufs=3) as dpool,
            tc.tile_pool(name="acts", bufs=2) as apool,
            tc.tile_pool(name="cp", bufs=8) as cppool,
            tc.tile_pool(name="small", bufs=1) as spool,
            tc.tile_pool(name="psg", bufs=1, space="PSUM") as ppg,
            tc.tile_pool(name="psm", bufs=7, space="PSUM") as ppm,
        ):
            # gate weights first on the sync ring (mem stream queues
            # behind); 2 packed constant tensors on the scalar ring.
            wts = []
            for h in range(8):
                wt_sb = wpool.tile([128, 4 * D_SH], f8, tag=f"wt{h}")
                nc.sync.dma_start(
                    out=wt_sb[:],
                    in_=wtp[:, h * 4 * D_SH : (h + 1) * 4 * D_SH],
                )
                wts.append(wt_sb)
            c8_sb = spool.tile([128, 35], f8, tag="c8")
            nc.scalar.dma_start(out=c8_sb[:], in_=c8[:])
            c32_sb = spool.tile([128, 10], f32, tag="c32")
            nc.scalar.dma_start(out=c32_sb[:], in_=c32[:])
            qc_sb = c8_sb[:, 0:32]
            ones_sb = c8_sb[:, 32:33]
            neg2_sb = c8_sb[:, 33:34]
            selsum8 = c8_sb[:, 34:35]
            b4 = c32_sb[:, 0:4]
            ones32_sb = c32_sb[:, 4:5]
            id4 = c32_sb[0:4, 5:9]
            selsum = c32_sb[:, 9:10]
            # preload Sigmoid+Abs activation tables off the critical path
            dum = spool.tile([1, 4], f32, tag="dum")
            nc.scalar.activation(dum[:], c32_sb[0:1, 0:4], AF.Sigmoid)
            nc.scalar.activation(dum[:], c32_sb[0:1, 0:4], AF.Abs)

            # ---- gate: 4 quadrant strips accumulate partial z rows ----
            zps = ppg.tile([128, D_SH], f32, tag="z")
            # zero the bank: zcp/selsum read all 128 partitions and stale
            # psum bits can be NaN (NaN*0 = NaN would poison the gate)
            nc.vector.memset(zps[:], 0.0)
            for j in range(32):
                r = j % 4
                nc.tensor.matmul(
                    zps[32 * r : 32 * r + 1, :],
                    qc_sb[:, j : j + 1],
                    wts[j // 4][:, (j % 4) * D_SH : (j % 4 + 1) * D_SH],
                    start=(j < 4),
                    stop=(j >= 28),
                    tile_position=(0, 32 * r),
                    skip_group_check=True,
                )
            hp = tc.high_priority()
            hp.__enter__()
            zcp = spool.tile([128, D_SH], mybir.dt.float16, tag="zcp")
            nc.scalar.activation(zcp[:], zps[:], AF.Copy)
            # strip-sum on the PE (selsum picks rows {0,32,64,96}), then
            # 4 row->column transposes; no SBUF<->SBUF DMA hop
            nc.tensor.matmul(
                zps[0:1, :], selsum8, zcp[:], start=True, stop=True,
                skip_group_check=True,
            )
            zrow = spool.tile([1, D_SH], f32, tag="zrow")
            nc.vector.tensor_copy(zrow[:], zps[0:1, :])
            tps = zps[:, 504:508]
            for c in range(DC):
                nc.tensor.transpose(
                    tps[:, c : c + 1],
                    zrow[0:1, c * 128 : (c + 1) * 128],
                    ones32_sb[0:1, 0:1],
                )
            zb = spool.tile([128, DC], f32, tag="zb")
            nc.vector.tensor_tensor(zb[:], tps, b4, A.add)
            gpos = spool.tile([128, DC], f32, tag="gpos")
            nc.scalar.activation(gpos[:], zb[:], AF.Sigmoid)
            negg = spool.tile([128, DC], f32, tag="negg")
            nc.vector.tensor_scalar(negg[:], gpos[:], -1.0, None, A.mult)
            hp.__exit__(None, None, None)

            # ---- main loop ----
            # phase 1 issues the gate-independent m-term matmuls right at
            # each tile's DMA: the PE consumes them in the idle gate
            # window, never overlapping the DVE/ScalarE pass on the same
            # tile (same-tile SBUF contention costs ~20%).
            for k in range(NK):
                bank0 = ppm.tile([128, 512], f32, tag="bank")
                bank1 = ppm.tile([128, 512], f32, tag="bank")
                banks = [bank0, bank1]
                total_passes = sum(
                    1 if (k, c) in ACT_TILES else 2 for c in range(DC)
                )
                seen = [0] * NG
                mts = []
                for c in range(DC):
                    if (k, c) in F16_TILES:
                        i16 = F16_TILES.index((k, c))
                        mt = mpool.tile(
                            [128, NT], mybir.dt.float16, tag="m16"
                        )
                        nc.sync.dma_start(
                            out=mt[:],
                            in_=memT16[i16 * 128 : (i16 + 1) * 128, :],
                        )
                    else:
                        mt = mpool.tile([128, NT], f8, tag="m")
                        nc.sync.dma_start(
                            out=mt[:],
                            in_=memT[
                                c * 128 : (c + 1) * 128,
                                k * NT : (k + 1) * NT,
                            ],
                        )
                    mts.append(mt)
                    if (k, c) not in ACT_TILES:
                        for j in range(NG):
                            nc.tensor.matmul(
                                banks[j // 4][32 * (j % 4) : 32 * (j % 4) + 1, :],
                                ones_sb,
                                mt[:, j * 512 : (j + 1) * 512],
                                start=(seen[j] == 0),
                                stop=(seen[j] == total_passes - 1),
                                tile_position=(0, 32 * (j % 4)),
                                skip_group_check=True,
                            )
                            seen[j] += 1
                for c in range(DC):
                    mt = mts[c]
                    if (k, c) in ACT_TILES:
                        at = apool.tile([128, NT], f8, tag="a")
                        nc.scalar.activation(
                            at[:], mt[:], AF.Abs, bias=negg[:, c : c + 1]
                        )
                        src_, stat = at, ones_sb
                    else:
                        dt = dpool.tile(
                            [128, NT],
                            mybir.dt.float16 if (k, c) in F16_TILES else f8,
                            tag="d16" if (k, c) in F16_TILES else "d",
                        )
                        hv = 2 if (k, c) == (NK - 1, DC - 1) else 1
                        for v in range(hv):
                            sl_v = slice(v * NT // hv, (v + 1) * NT // hv)
                            nc.vector.tensor_scalar(
                                dt[:, sl_v], mt[:, sl_v],
                                gpos[:, c : c + 1], 0.0,
                                A.subtract, A.min,
                            )
                        src_, stat = dt, neg2_sb
                    for j in range(NG):
                        nc.tensor.matmul(
                            banks[j // 4][32 * (j % 4) : 32 * (j % 4) + 1, :],
                            stat,
                            src_[:, j * 512 : (j + 1) * 512],
                            start=(seen[j] == 0),
                            stop=(seen[j] == total_passes - 1),
                            tile_position=(0, 32 * (j % 4)),
                            skip_group_check=True,
                        )
                        seen[j] += 1
                for h in range(2):
                    cp = cppool.tile([128, 512], f32, tag="cp")
                    if h == 0:
                        nc.vector.tensor_copy(cp[:], banks[h][:])
                    else:
                        nc.scalar.activation(cp[:], banks[h][:], AF.Copy)
                    nc.gpsimd.dma_start(
                        out=outp[8 * k + 4 * h : 8 * k + 4 * h + 4, :],
                        in_=cp[0:128:32, :],
                    )

            # per-d-chunk gate sums for the host-side m-term correction;
            # issued last so the PE queue never stalls on gpos mid-loop
            gs = zps[0:1, 500:504]
            nc.tensor.matmul(
                gs, ones32_sb, gpos[:], start=True, stop=True,
                skip_group_check=True,
            )
            gs_sb = spool.tile([1, DC], f32, tag="gs")
            nc.vector.tensor_copy(gs_sb[:], gs)
            nc.gpsimd.dma_start(out=outp[32:33, 0:DC], in_=gs_sb[:])

    nc.compile()
    return nc


def _get_nc():
    if "nc" not in _CACHE:
        _CACHE["nc"] = _build()
    return _CACHE["nc"]


def kernel(query, W, b, memory, _trace=False, _return_raw=False):
    f8 = ml_dtypes.float8_e3m4
    query = np.asarray(query, dtype=np.float32)
    W = np.asarray(W, dtype=np.float32)
    b = np.asarray(b, dtype=np.float32)
    memory = np.asarray(memory, dtype=np.float32)

    mem8T = np.ascontiguousarray(memory.astype(f8).T)       # [D, N] fp8
    W8 = W.astype(f8)
    q8 = query.reshape(32, 128).astype(f8).T                # [128, 32]
    c8 = np.zeros((128, 35), dtype=f8)
    c8[:, 0:32] = q8
    c8[:, 32] = f8(1.0)
    c8[:, 33] = f8(-2.0)
    c8[0:128:32, 34] = f8(1.0)

    in_maps = []
    for c in range(N_CORES):
        sl = slice(c * D_SH, (c + 1) * D_SH)
        # wtp[p, j*512 + n] = W.T[j*128 + p, n] = W8[sl][n, j*128+p]
        wsh = W8[sl, :]                       # [512, 4096]
        wtp = np.ascontiguousarray(
            wsh.T.reshape(32, 128, D_SH).transpose(1, 0, 2).reshape(128, -1)
        )
        c32 = np.zeros((128, 10), dtype=np.float32)
        c32[:, 0:4] = b[sl].reshape(4, 128).T
        c32[:, 4] = 1.0
        c32[0:4, 5:9] = np.eye(4, dtype=np.float32)
        c32[0:128:32, 9] = 1.0
        m16 = np.empty((len(F16_TILES) * 128, NT), dtype=np.float16)
        for i16, (tk, tc) in enumerate(F16_TILES):
            m16[i16 * 128 : (i16 + 1) * 128, :] = memory[
                tk * NT : (tk + 1) * NT,
                c * D_SH + tc * 128 : c * D_SH + (tc + 1) * 128,
            ].T.astype(np.float16)
        in_maps.append(
            {
                "memT": np.ascontiguousarray(mem8T[sl, :]),
                **({"memT16": m16} if F16_TILES else {}),
                "wtp": wtp,
                "c8": c8,
                "c32": c32,
            }
        )

    nc = _get_nc()
    res = run_bass_kernel_spmd(
        nc, in_maps, list(range(N_CORES)), trace=_trace
    )

    total = np.zeros(N, dtype=np.float64)
    for c in range(N_CORES):
        out = res.results[c]["outp"]
        gsum = out[32, 0:DC].astype(np.float64)   # sum of g per d-chunk
        rows = out[0:32].reshape(NK, NG, 512)
        corr = np.array(
            [sum(gsum[ci] for ci in DVE_SETS[k]) for k in range(NK)]
        )
        total += (rows - corr[:, None, None]).reshape(N)
    sims = (1.0 - total / D).astype(np.float32)
    mask = sims >= THRESHOLD
    if _return_raw:
        return (sims, mask), res
    return sims, mask


# revision 27
# speedup vs baseline: 1.0926x; 1.0371x over previous
"""Trainium2 Bass kernel for nn_BinaryMemory (retrieval_knn).

reference:
    gated = sigmoid(query @ W.T + b)                      # [1, D], D=4096
    sims  = 1 - mean(|memory - gated|, axis=-1)           # [N],   N=16384
    mask  = sims >= 0.8

Sharding (8 cores, no collectives): shard the D axis; core c owns
d-chunk [c*512, (c+1)*512). All bulk tensors stream as fp8_e3m4 (1 byte,
4 mantissa bits; operands live in (0,1) or N(0,1), quantization puts
~1e-2 relative on sims vs the 2e-2 budget). Per-core HBM ~10.1 MB; one
HWDGE ring sustains ~320 GB/s (a second ring adds nothing - per-core
HBM limit), so DMA floors the kernel at ~33 us + init.

Layout is d-on-partitions (memory shard transposed host-side to
[512 d, 16384 n]) so the gate value g[d] is a per-partition scalar.
The elementwise work splits |m-g| = (m-g) - 2*min(m-g, 0):
  - DVE tiles: ONE stock pass, tensor_scalar(op0=subtract scalar1=g,
    op1=min scalar2=0) -> min-term (fp8 in/out hits the 2x port mode,
    ~2.26 us per [128,4096]; a fused sub+abs is not expressible: the
    ISA rejects abs_max/bitwise op1 pairings and accum_out forces 1x).
  - ScalarE tiles: one Abs activation with per-partition bias -g
    (|m-g| directly, ~3.6 us/tile, dtype-agnostic).
The n-reduction runs on the otherwise-idle PE: per 512-column group a
psum row accumulates ones^T @ m (issued as soon as the tile lands - no
gate dependency) plus (-2*ones)^T @ minterm for DVE tiles, or
ones^T @ |m-g| for ScalarE tiles. The spurious +sum(g) from the m-term
is cancelled on the host via per-d-chunk gate sums (psum row 32 of the
output). Each psum bank holds 4 group-rows at quadrant offsets
{0,32,64,96} (tile_position); one [128,512] psum->SBUF copy drains 4
groups and a partition-strided DMA writes the rows to DRAM on the
otherwise-idle SWDGE (gpsimd) ring.

Gate: host pre-packs W-shard^T as [128, 32*512] fp8, DMA'd first on the
sync ring in 4 chunk-tiles so matmuls start as chunks land. q rides as
32 stationary columns; matmul j targets quadrant strip 32*(j%4)
(tile_position) so the PE's reorder window pulls each LDWEIGHTS ahead
of the in-flight MATMUL on the neighbouring strip - the serial LDW+MM
turnaround (~300 ns) drops to pipelined rate. The 4 partial z-rows are
copied out of psum, re-scattered to per-partition columns by a
transposed-AP SBUF->SBUF DMA, summed on DVE ([128,4,4] X-reduce),
biased with b, sigmoided on [128,4], and negated - no PE transpose, no
[1,512] row ops. Activation tables preload via dummy ops at t0, and
the small constants ride in two packed dram tensors (2 descriptors so
the scalar ring frees up early).
"""
import sys

sys.path.insert(0, "/opt/trn_rl_repo")

import numpy as np
import ml_dtypes

import concourse.bacc as bacc
import concourse.mybir as mybir
import concourse.tile as tile
from concourse.bass_utils import run_bass_kernel_spmd

N_CORES = 8
D = 4096
N = 16384
D_SH = D // N_CORES            # 512 dims per core
DC = D_SH // 128               # 4 d-chunks (partition blocks)
NT = 4096                      # n per tile
NK = N // NT                   # 4 n-chunks
NG = NT // 512                 # 8 psum groups per tile
THRESHOLD = 0.8
# tiles (k, c) computed on ScalarE via Abs; the rest on DVE via sub+min
ACT_TILES = {(0, 2), (1, 1), (1, 3), (2, 0), (3, 1), (3, 2)}
F16_TILES = []   # fp16 variants measured slower: DMA is too tight
DVE_SETS = [
    [c for c in range(DC) if (k, c) not in ACT_TILES] for k in range(NK)
]

_CACHE = {}


def _build():
    f32 = mybir.dt.float32
    f8 = mybir.dt.float8e3
    A = mybir.AluOpType
    AF = mybir.ActivationFunctionType
    nc = bacc.Bacc(
        "TRN2", target_bir_lowering=False, debug=False, num_devices=N_CORES
    )

    memT = nc.dram_tensor("memT", [D_SH, N], f8, kind="ExternalInput")
    memT16 = (
        nc.dram_tensor(
            "memT16", [len(F16_TILES) * 128, NT], mybir.dt.float16,
            kind="ExternalInput",
        )
        if F16_TILES
        else None
    )
    # W shard, host-packed: partition p, chunk j holds W.T[j*128 + p, :]
    wtp = nc.dram_tensor("wtp", [128, 32 * D_SH], f8, kind="ExternalInput")
    # packed constants: cols 0:32 qcol, 32 ones, 33 neg2, 34 selsum
    c8 = nc.dram_tensor("c8", [128, 35], f8, kind="ExternalInput")
    # packed f32 constants: cols 0:4 b columns, 4 ones, 5:9 eye(4),
    # 9 selsum (1.0 at partitions {0,32,64,96})
    c32 = nc.dram_tensor("c32", [128, 10], f32, kind="ExternalInput")
    outp = nc.dram_tensor("outp", [33, 512], f32, kind="ExternalOutput")

    with tile.TileContext(nc) as tc:
        with (
            tc.tile_pool(name="wts", bufs=1) as wpool,
            tc.tile_pool(name="mem", bufs=10) as mpool,
            tc.tile_pool(name="dts", bufs=3) as dpool,
            tc.tile_pool(name="acts", bufs=2) as apool,
            tc.tile_pool(name="cp", bufs=8) as cppool,
            tc.tile_pool(name="small", bufs=1) as spool,
            tc.tile_pool(name="psg", bufs=1, space="PSUM") as ppg,
            tc.tile_pool(name="psm", bufs=7, space="PSUM") as ppm,
        ):
            # gate weights first on the sync ring (mem stream queues
            # behind); 2 packed constant tensors on the scalar ring.
            wts = []
            for h in range(8):
                wt_sb = wpool.tile([128, 4 * D_SH], f8, tag=f"wt{h}")
                nc.sync.dma_start(
                    out=wt_sb[:],
                    in_=wtp[:, h * 4 * D_SH : (h + 1) * 4 * D_SH],
                )
                wts.append(wt_sb)
            c8_sb = spool.tile([128, 35], f8, tag="c8")
            nc.scalar.dma_start(out=c8_sb[:], in_=c8[:])
            c32_sb = spool.tile([128, 10], f32, tag="c32")
            nc.scalar.dma_start(out=c32_sb[:], in_=c32[:])
            qc_sb = c8_sb[:, 0:32]
            ones_sb = c8_sb[:, 32:33]
            neg2_sb = c8_sb[:, 33:34]
            selsum8 = c8_sb[:, 34:35]
            b4 = c32_sb[:, 0:4]
            ones32_sb = c32_sb[:, 4:5]
            id4 = c32_sb[0:4, 5:9]
            selsum = c32_sb[:, 9:10]
            # preload Sigmoid+Abs activation tables off the critical path
            dum = spool.tile([1, 4], f32, tag="dum")
            nc.scalar.activation(dum[:], c32_sb[0:1, 0:4], AF.Sigmoid)
            nc.scalar.activation(dum[:], c32_sb[0:1, 0:4], AF.Abs)

            # ---- gate: 4 quadrant strips accumulate partial z rows ----
            zps = ppg.tile([128, D_SH], f32, tag="z")
            # zero the bank: zcp/selsum read all 128 partitions and stale
            # psum bits can be NaN (NaN*0 = NaN would poison the gate)
            nc.vector.memset(zps[:], 0.0)
            for j in range(32):
                r = j % 4
                nc.tensor.matmul(
                    zps[32 * r : 32 * r + 1, :],
                    qc_sb[:, j : j + 1],
                    wts[j // 4][:, (j % 4) * D_SH : (j % 4 + 1) * D_SH],
                    start=(j < 4),
                    stop=(j >= 28),
                    tile_position=(0, 32 * r),
                    skip_group_check=True,
                )
            hp = tc.high_priority()
            hp.__enter__()
            zcp = spool.tile([128, D_SH], mybir.dt.float16, tag="zcp")
            nc.scalar.activation(zcp[:], zps[:], AF.Copy)
            # strip-sum on the PE (selsum picks rows {0,32,64,96}), then
            # 4 row->column transposes; no SBUF<->SBUF DMA hop
            nc.tensor.matmul(
                zps[0:1, :], selsum8, zcp[:], start=True, stop=True,
                skip_group_check=True,
            )
            zrow = spool.tile([1, D_SH], f32, tag="zrow")
            nc.vector.tensor_copy(zrow[:], zps[0:1, :])
            tps = zps[:, 504:508]
            for c in range(DC):
                nc.tensor.transpose(
                    tps[:, c : c + 1],
                    zrow[0:1, c * 128 : (c + 1) * 128],
                    ones32_sb[0:1, 0:1],
                )
            zb = spool.tile([128, DC], f32, tag="zb")
            nc.vector.tensor_tensor(zb[:], tps, b4, A.add)
            gpos = spool.tile([128, DC], f32, tag="gpos")
            nc.scalar.activation(gpos[:], zb[:], AF.Sigmoid)
            negg = spool.tile([128, DC], f32, tag="negg")
            nc.vector.tensor_scalar(negg[:], gpos[:], -1.0, None, A.mult)
            hp.__exit__(None, None, None)

            # ---- main loop ----
            # phase 1 issues the gate-independent m-term matmuls right at
            # each tile's DMA: the PE consumes them in the idle gate
            # window, never overlapping the DVE/ScalarE pass on the same
            # tile (same-tile SBUF contention costs ~20%).
            for k in range(NK):
                bank0 = ppm.tile([128, 512], f32, tag="bank")
                bank1 = ppm.tile([128, 512], f32, tag="bank")
                banks = [bank0, bank1]
                total_passes = sum(
                    1 if (k, c) in ACT_TILES else 2 for c in range(DC)
                )
                seen = [0] * NG
                mts = []
                for c in range(DC):
                    if (k, c) in F16_TILES:
                        i16 = F16_TILES.index((k, c))
                        mt = mpool.tile(
                            [128, NT], mybir.dt.float16, tag="m16"
                        )
                        nc.sync.dma_start(
                            out=mt[:],
                            in_=memT16[i16 * 128 : (i16 + 1) * 128, :],
                        )
                    else:
                        mt = mpool.tile([128, NT], f8, tag="m")
                        nc.sync.dma_start(
                            out=mt[:],
                            in_=memT[
                                c * 128 : (c + 1) * 128,
                                k * NT : (k + 1) * NT,
                            ],
                        )
                    mts.append(mt)
                    if (k, c) not in ACT_TILES:
                        for j in range(NG):
                            nc.tensor.matmul(
                                banks[j // 4][32 * (j % 4) : 32 * (j % 4) + 1, :],
                                ones_sb,
                                mt[:, j * 512 : (j + 1) * 512],
                                start=(seen[j] == 0),
                                stop=(seen[j] == total_passes - 1),
                                tile_position=(0, 32 * (j % 4)),
                                skip_group_check=True,
                            )
                            seen[j] += 1
                for c in range(DC):
                    mt = mts[c]
                    if (k, c) in ACT_TILES:
                        at = apool.tile([128, NT], f8, tag="a")
                        nc.scalar.activation(
                            at[:], mt[:], AF.Abs, bias=negg[:, c : c + 1]
                        )
                        src_, stat = at, ones_sb
                    else:
                        dt = dpool.tile(
                            [128, NT],
                            mybir.dt.float16 if (k, c) in F16_TILES else f8,
                            tag="d16" if (k, c) in F16_TILES else "d",
                        )
                        hv = 2 if (k, c) == (NK - 1, DC - 1) else 1
                        for v in range(hv):
                            sl_v = slice(v * NT // hv, (v + 1) * NT // hv)
                            nc.vector.tensor_scalar(
                                dt[:, sl_v], mt[:, sl_v],
                                gpos[:, c : c + 1], 0.0,
                                A.subtract, A.min,
                            )
                        src_, stat = dt, neg2_sb
                    for j in range(NG):
                        nc.tensor.matmul(
                            banks[j // 4][32 * (j % 4) : 32 * (j % 4) + 1, :],
                            stat,
                            src_[:, j * 512 : (j + 1) * 512],
                            start=(seen[j] == 0),
                            stop=(seen[j] == total_passes - 1),
                            tile_position=(0, 32 * (j % 4)),
                            skip_group_check=True,
                        )
                        seen[j] += 1
                for h in range(2):
                    cp = cppool.tile([128, 512], f32, tag="cp")
                    if h == 0:
                        nc.vector.tensor_copy(cp[:], banks[h][:])
                    else:
                        nc.scalar.activation(cp[:], banks[h][:], AF.Copy)
                    nc.gpsimd.dma_start(
                        out=outp[8 * k + 4 * h : 8 * k + 4 * h + 4, :],
                        in_=cp[0:128:32, :],
                    )

            # per-d-chunk gate sums for the host-side m-term correction;
            # issued last so the PE queue never stalls on gpos mid-loop
            gs = zps[0:1, 500:504]
            nc.tensor.matmul(
                gs, ones32_sb, gpos[:], start=True, stop=True,
                skip_group_check=True,
            )
            gs_sb = spool.tile([1, DC], f32, tag="gs")
            nc.vector.tensor_copy(gs_sb[:], gs)
            nc.gpsimd.dma_start(out=outp[32:33, 0:DC], in_=gs_sb[:])

    nc.compile()
    return nc


def _get_nc():
    if "nc" not in _CACHE:
        _CACHE["nc"] = _build()
    return _CACHE["nc"]


def kernel(query, W, b, memory, _trace=False, _return_raw=False):
    f8 = ml_dtypes.float8_e3m4
    query = np.asarray(query, dtype=np.float32)
    W = np.asarray(W, dtype=np.float32)
    b = np.asarray(b, dtype=np.float32)
    memory = np.asarray(memory, dtype=np.float32)

    mem8T = np.ascontiguousarray(memory.astype(f8).T)       # [D, N] fp8
    W8 = W.astype(f8)
    q8 = query.reshape(32, 128).astype(f8).T                # [128, 32]
    c8 = np.zeros((128, 35), dtype=f8)
    c8[:, 0:32] = q8
    c8[:, 32] = f8(1.0)
    c8[:, 33] = f8(-2.0)
    c8[0:128:32, 34] = f8(1.0)

    in_maps = []
    for c in range(N_CORES):
        sl = slice(c * D_SH, (c + 1) * D_SH)
        # wtp[p, j*512 + n] = W.T[j*128 + p, n] = W8[sl][n, j*128+p]
        wsh = W8[sl, :]                       # [512, 4096]
        wtp = np.ascontiguousarray(
            wsh.T.reshape(32, 128, D_SH).transpose(1, 0, 2).reshape(128, -1)
        )
        c32 = np.zeros((128, 10), dtype=np.float32)
        c32[:, 0:4] = b[sl].reshape(4, 128).T
        c32[:, 4] = 1.0
        c32[0:4, 5:9] = np.eye(4, dtype=np.float32)
        c32[0:128:32, 9] = 1.0
        m16 = np.empty((len(F16_TILES) * 128, NT), dtype=np.float16)
        for i16, (tk, tc) in enumerate(F16_TILES):
            m16[i16 * 128 : (i16 + 1) * 128, :] = memory[
                tk * NT : (tk + 1) * NT,
                c * D_SH + tc * 128 : c * D_SH + (tc + 1) * 128,
            ].T.astype(np.float16)
        in_maps.append(
            {
                "memT": np.ascontiguousarray(mem8T[sl, :]),
                **({"memT16": m16} if F16_TILES else {}),
                "wtp": wtp,
                "c8": c8,
                "c32": c32,
            }
        )

    nc = _get_nc()
    res = run_bass_kernel_spmd(
        nc, in_maps, list(range(N_CORES)), trace=_trace
    )

    total = np.zeros(N, dtype=np.float64)
    for c in range(N_CORES):
        out = res.results[c]["outp"]
        gsum = out[32, 0:DC].astype(np.float64)   # sum of g per d-chunk
        rows = out[0:32].reshape(NK, NG, 512)
        corr = np.array(
            [sum(gsum[ci] for ci in DVE_SETS[k]) for k in range(NK)]
        )
        total += (rows - corr[:, None, None]).reshape(N)
    sims = (1.0 - total / D).astype(np.float32)
    mask = sims >= THRESHOLD
    if _return_raw:
        return (sims, mask), res
    return sims, mask
